# revision 19
# baseline (speedup 1.0000x reference)
"""Masked phase-locking value (PLV) kernel for Trainium2, 8 NeuronCores.

Math: out[b] = |sum_ij M_ij * exp(i*(a_bi - b_bj))| / max(sum(M), 1)
    real_b = sum_ij M_ij (cos a_bi cos b_bj + sin a_bi sin b_bj)
    imag_b = sum_ij M_ij (sin a_bi cos b_bj - cos a_bi sin b_bj)

Device decomposition (per core, Na sharded 8 ways -> NI=1024 rows each):
    Y[m, i] = sum_j V[j, m] * M[i, j]      (TensorE; V = [cb^T | sb^T], m = 2B = 128)
The tiny final reduce racc[m] = sum_i Y[m, i] * U[m, i] runs on the HOST
(2M bf16 MACs total): an on-device fused-reduce epilogue costs ~4us of
serial DVE time that cannot hide behind the last matmuls, while shipping
Y as bf16 (256KB/core) costs ~1us and removes the U stream entirely.

The mask is binary, so 4 mask elements ride in each fp8 byte as BIT PLANES
(bits 0x08/0x10/0x20/0x40 = exact fp8e4 values 2^-6/2^-5/2^-3/2.0):
  - HBM mask traffic drops 8MB -> 2MB per core; the whole input stream is
    2MB mask + 1MB trig weights = 3MB (~8.5us at 358GB/s).
  - on-device extraction = one DVE tensor_scalar(bitwise_and) per
    (chunk, plane) on uint32-punned data -> byte-exact fp8 plane tensors.
    uint32 runs in 2x_2P mode (8 bytes/cycle/lane): ~1.35us/chunk, under
    the PE's 1.74us/chunk consumption rate.
  - per-plane scale 2^k folds into the fp8 weights (|w| <= 64 < 240 max),
    so precision matches an unpacked fp8 kernel.
The PE runs 8 chunks x 4 planes x 4 banks = 128 DoubleRow matmuls
(contraction 256/instr, FD=256) = the 13.7us fp8 roofline; it is the
pacing engine, everything else hides under it.

Schedule: ALL DMA rides the sync ring in exact consumption order
(v/pk interleaved) — a second ring would round-robin at the SDMA level
and skew arrivals. 14 warm-up matmuls bridge the HAM clock ramp so the
real stream starts at 2.4GHz. The 4 PSUM accumulation regions live in one
[128, 4, 512] tile (4 banks); after the last stop-matmul a single DVE
tensor_copy downcasts all of Y to bf16 and one 256KB DMA ships it out.
"""

import numpy as np

import concourse.bass as bass
import concourse.tile as tile
from concourse import bacc, mybir
from concourse.bass_utils import run_bass_kernel_spmd

B = 64
NA = 8192
NB = 8192
NCORES = 8
NI = NA // NCORES            # mask rows (i) per core
NPL = 4                      # mask bit-planes packed per byte
NJB = NB // NPL              # 2048 packed bytes per mask row
KC = NJB // 256              # 8 contraction chunks of 256 bytes
TK = 2 * KC                  # tile dim1: t = 2*kb + q (DoubleRow pair slot q)
NIW = NI // 4                # uint32 words per (partition, t) row
MMSL = 256                   # matmul i-slice (FD); each bank owns a full PSUM bank
NBK = NI // MMSL             # 4 accumulation banks
NWU = 24                     # PE warm-up matmuls: bridge the HAM clock ramp
                             # AND the first-chunk DMA latency (~5us) so the
                             # real stream enters at 2.4GHz with no idle gap
BITS = [0x08, 0x10, 0x20, 0x40]
BITVAL = [2.0 ** -6, 2.0 ** -5, 2.0 ** -3, 2.0]
ANDMASK = [b * 0x01010101 for b in BITS]

F8 = mybir.dt.float8e4
U32 = mybir.dt.uint32
F32 = mybir.dt.float32
BF16 = mybir.dt.bfloat16


def build_program() -> bass.Bass:
    nc = bacc.Bacc("TRN2")
    # host layouts are p-major: dim0 = SBUF partition, per-partition contiguous
    pk_d = nc.dram_tensor("pk", [128, TK, NIW], U32, kind="ExternalInput")
    v_d = nc.dram_tensor("v", [128, KC, NPL, 2, 2 * B], F8, kind="ExternalInput")
    out_d = nc.dram_tensor("y", [128, NI], BF16, kind="ExternalOutput")

    DR = mybir.MatmulPerfMode.DoubleRow
    band = mybir.AluOpType.bitwise_and

    with tile.TileContext(nc) as tc:
        with (
            tc.tile_pool(name="consts", bufs=1) as consts,
            tc.tile_pool(name="psum", bufs=1, space="PSUM") as psum_pool,
        ):
            jw = consts.tile([128, 2, MMSL], F8)
            nc.vector.memset(jw, 0)
            pk_sb = consts.tile([128, TK, NIW], U32)
            pl_sb = consts.tile([128, NPL, TK, NIW], U32)
            v_sb = consts.tile([128, KC, NPL, 2, 2 * B], F8)
            yb = consts.tile([128, NI], BF16)

            # DMA plan: packed mask on the sync ring, weights on the scalar
            # ring, each in consumption order. The SDMA engines round-robin
            # between the two rings at packet granularity, so each ring's
            # per-block completion ceremony (~1us receipt stall) hides under
            # the other ring's stream; a single ring pays it serially.
            nc.sync.dma_start(out=pk_sb[:, 0:2], in_=pk_d[:, 0:2])
            nc.sync.dma_start(out=pk_sb[:, 2:4], in_=pk_d[:, 2:4])
            nc.sync.dma_start(out=pk_sb[:, 4:8], in_=pk_d[:, 4:8])
            nc.sync.dma_start(out=pk_sb[:, 8:12], in_=pk_d[:, 8:12])
            nc.scalar.dma_start(out=v_sb[:, 0:2], in_=v_d[:, 0:2])
            nc.scalar.dma_start(out=v_sb[:, 2:8], in_=v_d[:, 2:8])
            nc.scalar.dma_start(out=pk_sb[:, 12:16], in_=pk_d[:, 12:16])

            # one tile per bank-PAIR (2 whole PSUM banks each): the first
            # pair's downcast copy must not serialize the second pair's
            # final matmuls, which a single merged tile would force
            psp = [
                psum_pool.tile([128, NBK // 2, 512], F32, name=f"ps{i}")
                for i in range(2)
            ]
            wu = psum_pool.tile([128, 512], F32)

            # PE warm-up on junk (no DMA dependency) to beat the clock ramp
            for r in range(NWU):
                nc.tensor.matmul(
                    out=wu[:, 0:MMSL], lhsT=jw[:, :, 0:128], rhs=jw[:],
                    start=(r == 0), stop=(r == NWU - 1), perf_mode=DR,
                )

            # plane extraction: bitwise AND on uint32-punned bytes; emitted
            # in consumption order so the DVE FIFO matches the PE's needs.
            # The very first AND covers only bank 0's i-range so the first
            # real matmul unblocks ~0.25us sooner.
            for kb in range(KC):
                tsl = slice(2 * kb, 2 * kb + 2)
                for k in range(NPL):
                    if kb == 0 and k == 0:
                        nc.vector.tensor_scalar(
                            out=pl_sb[:, 0, tsl, 0:64], in0=pk_sb[:, tsl, 0:64],
                            scalar1=ANDMASK[0], scalar2=None, op0=band,
                        )
                        nc.vector.tensor_scalar(
                            out=pl_sb[:, 0, tsl, 64:256], in0=pk_sb[:, tsl, 64:256],
                            scalar1=ANDMASK[0], scalar2=None, op0=band,
                        )
                        continue
                    nc.vector.tensor_scalar(
                        out=pl_sb[:, k, tsl], in0=pk_sb[:, tsl],
                        scalar1=ANDMASK[k], scalar2=None, op0=band,
                    )

            def rhs(kb, k, sb):
                return pl_sb[
                    :, k, 2 * kb : 2 * kb + 2, 64 * sb : 64 * (sb + 1)
                ].bitcast(F8)

            for kb in range(KC - 1):
                for k in range(NPL):
                    lhsT = v_sb[:, kb, k]
                    for sb in range(NBK):
                        nc.tensor.matmul(
                            out=psp[sb // 2][:, sb % 2, 0:MMSL],
                            lhsT=lhsT,
                            rhs=rhs(kb, k, sb),
                            start=(kb == 0 and k == 0),
                            stop=False,
                            perf_mode=DR,
                        )
            # last chunk runs in bank-PAIRS: two matmuls per weight load (the
            # load still hides), and each pair's fused downcast copy + 128KB
            # DMA-out pipelines under the other pair's matmuls
            kb = KC - 1
            for pr in range(2):
                for k in range(NPL):
                    lhsT = v_sb[:, kb, k]
                    for h in range(NBK // 2):
                        nc.tensor.matmul(
                            out=psp[pr][:, h, 0:MMSL],
                            lhsT=lhsT,
                            rhs=rhs(kb, k, 2 * pr + h),
                            start=False,
                            stop=(k == NPL - 1),
                            perf_mode=DR,
                        )
                ysl = slice(pr * 2 * MMSL, (pr + 1) * 2 * MMSL)
                nc.vector.tensor_copy(
                    yb[:, ysl].rearrange("p (s i) -> p s i", s=NBK // 2),
                    psp[pr][:, :, 0:MMSL],
                )
                eng = nc.sync if pr == 0 else nc.scalar
                eng.dma_start(out=out_d[:, ysl], in_=yb[:, ysl])
    nc.finalize()
    return nc


def prep_inputs(phases_a, phases_b, coupling_mask):
    f8np = mybir.dt.np(F8)
    pb = np.asarray(phases_b, dtype=np.float32)
    cb, sb = np.cos(pb), np.sin(pb)

    m_u8 = (np.asarray(coupling_mask) != 0).astype(np.uint8)

    # weights: V[p, kb, k, q, m] = T2[m, j]/BITVAL[k], j = 4*(256kb+2p+q)+k
    T2 = np.concatenate([cb, sb], axis=0)                      # [128 m, NB j]
    W = np.ascontiguousarray(T2.T)                             # [NB j, 128 m]
    W = W.reshape(KC, 128, 2, NPL, 128).transpose(1, 0, 3, 2, 4)
    W = W / np.asarray(BITVAL, np.float32)[None, None, :, None, None]
    v_host = W.astype(f8np)                                    # [128,KC,NPL,2,128]

    in_maps = []
    for c in range(NCORES):
        sl = slice(c * NI, (c + 1) * NI)
        A = m_u8[sl]                                           # [NI i, NB j]
        # pack 4 j's per byte at bits 3..6: byte[i, jb] = sum_k A[i,4jb+k]<<(3+k)
        A4 = A.reshape(NI, NJB, NPL)
        P = (
            (A4[:, :, 0] << 3) | (A4[:, :, 1] << 4)
            | (A4[:, :, 2] << 5) | (A4[:, :, 3] << 6)
        ).astype(np.uint8)                                     # [NI, NJB]
        pk_host = (
            np.ascontiguousarray(P.reshape(NI, KC, 128, 2).transpose(2, 1, 3, 0))
            .reshape(128, TK, NI)
            .view(np.uint32)
        )                                                      # [128, TK, NIW]
        in_maps.append({"pk": pk_host, "v": v_host})
    return in_maps


def combine(outs, phases_a, coupling_mask):
    pa = np.asarray(phases_a, dtype=np.float32)
    ca, sa = np.cos(pa), np.sin(pa)                            # [B, NA]
    real = np.zeros(B, np.float64)
    imag = np.zeros(B, np.float64)
    for c in range(NCORES):
        sl = slice(c * NI, (c + 1) * NI)
        y = np.asarray(outs[c]).astype(np.float32)             # [128 m, NI i]
        yt, yb_ = y[:B], y[B:]                                 # cb-part, sb-part
        cac, sac = ca[:, sl], sa[:, sl]                        # [B, NI]
        real += np.einsum('bi,bi->b', yt, cac, dtype=np.float64)
        real += np.einsum('bi,bi->b', yb_, sac, dtype=np.float64)
        imag += np.einsum('bi,bi->b', yt, sac, dtype=np.float64)
        imag -= np.einsum('bi,bi->b', yb_, cac, dtype=np.float64)
    n_pairs = max(float(np.count_nonzero(np.asarray(coupling_mask))), 1.0)
    return (np.sqrt(real * real + imag * imag) / n_pairs).astype(np.float32)


_prog_cache: list = []


def kernel(phases_a, phases_b, coupling_mask):
    in_maps = prep_inputs(phases_a, phases_b, coupling_mask)
    if not _prog_cache:
        _prog_cache.append(build_program())
    res = run_bass_kernel_spmd(_prog_cache[0], in_maps, core_ids=list(range(NCORES)))
    return combine([r["y"] for r in res.results], phases_a, coupling_mask)


# revision 21
# speedup vs baseline: 1.0442x; 1.0442x over previous
"""Masked phase-locking value (PLV) kernel for Trainium2, 8 NeuronCores.

Math: out[b] = |sum_ij M_ij * exp(i*(a_bi - b_bj))| / max(sum(M), 1)
    real_b = sum_ij M_ij (cos a_bi cos b_bj + sin a_bi sin b_bj)
    imag_b = sum_ij M_ij (sin a_bi cos b_bj - cos a_bi sin b_bj)

Device decomposition (per core, Na sharded 8 ways -> NI=1024 rows each):
    Y[m, i] = sum_j V[j, m] * M[i, j]      (TensorE; V = [cb^T | sb^T], m = 2B = 128)
The tiny final reduce racc[m] = sum_i Y[m, i] * U[m, i] runs on the HOST
(2M bf16 MACs total): an on-device fused-reduce epilogue costs ~4us of
serial DVE time that cannot hide behind the last matmuls, while shipping
Y as bf16 (256KB/core) costs ~1us and removes the U stream entirely.

The mask is binary, so 4 mask elements ride in each fp8 byte as BIT PLANES
(bits 0x08/0x10/0x20/0x40 = exact fp8e4 values 2^-6/2^-5/2^-3/2.0):
  - HBM mask traffic drops 8MB -> 2MB per core; the whole input stream is
    2MB mask + 1MB trig weights = 3MB (~8.5us at 358GB/s).
  - on-device extraction = one DVE tensor_scalar(bitwise_and) per
    (chunk, plane) on uint32-punned data -> byte-exact fp8 plane tensors.
    uint32 runs in 2x_2P mode (8 bytes/cycle/lane): ~1.35us/chunk, under
    the PE's 1.74us/chunk consumption rate.
  - per-plane scale 2^k folds into the fp8 weights (|w| <= 64 < 240 max),
    so precision matches an unpacked fp8 kernel.
The PE runs 8 chunks x 4 planes x 4 banks = 128 DoubleRow matmuls
(contraction 256/instr, FD=256) = the 13.7us fp8 roofline; it is the
pacing engine, everything else hides under it.

Schedule: ALL DMA rides the sync ring in exact consumption order
(v/pk interleaved) — a second ring would round-robin at the SDMA level
and skew arrivals. 14 warm-up matmuls bridge the HAM clock ramp so the
real stream starts at 2.4GHz. The 4 PSUM accumulation regions live in one
[128, 4, 512] tile (4 banks); after the last stop-matmul a single DVE
tensor_copy downcasts all of Y to bf16 and one 256KB DMA ships it out.
"""

import numpy as np

import concourse.bass as bass
import concourse.tile as tile
from concourse import bacc, mybir
from concourse.bass_utils import run_bass_kernel_spmd

B = 64
NA = 8192
NB = 8192
NCORES = 8
NI = NA // NCORES            # mask rows (i) per core
NPL = 4                      # mask bit-planes packed per byte
NJB = NB // NPL              # 2048 packed bytes per mask row
KC = NJB // 256              # 8 contraction chunks of 256 bytes
TK = 2 * KC                  # tile dim1: t = 2*kb + q (DoubleRow pair slot q)
NIW = NI // 4                # uint32 words per (partition, t) row
MMSL = 256                   # matmul i-slice (FD); each bank owns a full PSUM bank
NBK = NI // MMSL             # 4 accumulation banks
NWU = 24                     # PE warm-up matmuls: bridge the HAM clock ramp
                             # AND the first-chunk DMA latency (~5us) so the
                             # real stream enters at 2.4GHz with no idle gap
BITS = [0x08, 0x10, 0x20, 0x40]
BITVAL = [2.0 ** -6, 2.0 ** -5, 2.0 ** -3, 2.0]
ANDMASK = [b * 0x01010101 for b in BITS]

F8 = mybir.dt.float8e4
U32 = mybir.dt.uint32
F32 = mybir.dt.float32
BF16 = mybir.dt.bfloat16


def build_program() -> bass.Bass:
    nc = bacc.Bacc("TRN2")
    # host layouts are p-major: dim0 = SBUF partition, per-partition contiguous
    pk_d = nc.dram_tensor("pk", [128, TK, NIW], U32, kind="ExternalInput")
    v_d = nc.dram_tensor("v", [128, KC, NPL, 2, 2 * B], F8, kind="ExternalInput")
    out_d = nc.dram_tensor("y", [128, NI], BF16, kind="ExternalOutput")

    DR = mybir.MatmulPerfMode.DoubleRow
    band = mybir.AluOpType.bitwise_and

    with tile.TileContext(nc) as tc:
        with (
            tc.tile_pool(name="consts", bufs=1) as consts,
            tc.tile_pool(name="psum", bufs=1, space="PSUM") as psum_pool,
        ):
            jw = consts.tile([128, 2, MMSL], F8)
            nc.vector.memset(jw, 0)
            pk_sb = consts.tile([128, TK, NIW], U32)
            pl_sb = consts.tile([128, NPL, TK, NIW], U32)
            v_sb = consts.tile([128, KC, NPL, 2, 2 * B], F8)
            yb = consts.tile([128, NI], BF16)

            # DMA plan: all QUEUED transfers stream concurrently on the SDMA
            # engines (no FIFO between blocks), so late blocks dilute the
            # bandwidth of the first chunks the PE is waiting on. Fix: a
            # two-PHASE schedule. Phase 1 queues only the critical blocks
            # (first two mask chunks + first weights) at full rate. A tiny
            # GATE DMA on each queue READS the first mask block, so its
            # trigger stalls at the sequencer until that block completes —
            # holding every phase-2 descriptor off the rings until then.
            gate = consts.tile([128, 2], U32)
            nc.sync.dma_start(out=pk_sb[:, 0:2], in_=pk_d[:, 0:2])
            nc.sync.dma_start(out=pk_sb[:, 2:6], in_=pk_d[:, 2:6])
            nc.scalar.dma_start(out=v_sb[:, 0:2], in_=v_d[:, 0:2])
            nc.sync.dma_start(out=gate[:, 0:1], in_=pk_sb[:, 0:1, 0:1])
            nc.scalar.dma_start(out=gate[:, 1:2], in_=pk_sb[:, 1:2, 0:1])
            nc.sync.dma_start(out=pk_sb[:, 6:10], in_=pk_d[:, 6:10])
            nc.sync.dma_start(out=pk_sb[:, 10:16], in_=pk_d[:, 10:16])
            nc.scalar.dma_start(out=v_sb[:, 2:4], in_=v_d[:, 2:4])
            nc.scalar.dma_start(out=v_sb[:, 4:8], in_=v_d[:, 4:8])

            # one tile per bank-PAIR (2 whole PSUM banks each): the first
            # pair's downcast copy must not serialize the second pair's
            # final matmuls, which a single merged tile would force
            psp = [
                psum_pool.tile([128, NBK // 2, 512], F32, name=f"ps{i}")
                for i in range(2)
            ]
            wu = psum_pool.tile([128, 512], F32)

            # PE warm-up on junk (no DMA dependency) to beat the clock ramp
            for r in range(NWU):
                nc.tensor.matmul(
                    out=wu[:, 0:MMSL], lhsT=jw[:, :, 0:128], rhs=jw[:],
                    start=(r == 0), stop=(r == NWU - 1), perf_mode=DR,
                )

            # plane extraction: bitwise AND on uint32-punned bytes; emitted
            # in consumption order so the DVE FIFO matches the PE's needs.
            # The very first AND covers only bank 0's i-range so the first
            # real matmul unblocks ~0.25us sooner.
            for kb in range(KC):
                tsl = slice(2 * kb, 2 * kb + 2)
                for k in range(NPL):
                    if kb == 0 and k == 0:
                        nc.vector.tensor_scalar(
                            out=pl_sb[:, 0, tsl, 0:64], in0=pk_sb[:, tsl, 0:64],
                            scalar1=ANDMASK[0], scalar2=None, op0=band,
                        )
                        nc.vector.tensor_scalar(
                            out=pl_sb[:, 0, tsl, 64:256], in0=pk_sb[:, tsl, 64:256],
                            scalar1=ANDMASK[0], scalar2=None, op0=band,
                        )
                        continue
                    nc.vector.tensor_scalar(
                        out=pl_sb[:, k, tsl], in0=pk_sb[:, tsl],
                        scalar1=ANDMASK[k], scalar2=None, op0=band,
                    )

            def rhs(kb, k, sb):
                return pl_sb[
                    :, k, 2 * kb : 2 * kb + 2, 64 * sb : 64 * (sb + 1)
                ].bitcast(F8)

            for kb in range(KC - 1):
                for k in range(NPL):
                    lhsT = v_sb[:, kb, k]
                    for sb in range(NBK):
                        nc.tensor.matmul(
                            out=psp[sb // 2][:, sb % 2, 0:MMSL],
                            lhsT=lhsT,
                            rhs=rhs(kb, k, sb),
                            start=(kb == 0 and k == 0),
                            stop=False,
                            perf_mode=DR,
                        )
            # last chunk runs in bank-PAIRS: two matmuls per weight load (the
            # load still hides), and each pair's fused downcast copy + 128KB
            # DMA-out pipelines under the other pair's matmuls
            kb = KC - 1
            for pr in range(2):
                for k in range(NPL):
                    lhsT = v_sb[:, kb, k]
                    for h in range(NBK // 2):
                        nc.tensor.matmul(
                            out=psp[pr][:, h, 0:MMSL],
                            lhsT=lhsT,
                            rhs=rhs(kb, k, 2 * pr + h),
                            start=False,
                            stop=(k == NPL - 1),
                            perf_mode=DR,
                        )
                ysl = slice(pr * 2 * MMSL, (pr + 1) * 2 * MMSL)
                nc.vector.tensor_copy(
                    yb[:, ysl].rearrange("p (s i) -> p s i", s=NBK // 2),
                    psp[pr][:, :, 0:MMSL],
                )
                eng = nc.sync if pr == 0 else nc.scalar
                eng.dma_start(out=out_d[:, ysl], in_=yb[:, ysl])
    nc.finalize()
    return nc


def prep_inputs(phases_a, phases_b, coupling_mask):
    f8np = mybir.dt.np(F8)
    pb = np.asarray(phases_b, dtype=np.float32)
    cb, sb = np.cos(pb), np.sin(pb)

    m_u8 = (np.asarray(coupling_mask) != 0).astype(np.uint8)

    # weights: V[p, kb, k, q, m] = T2[m, j]/BITVAL[k], j = 4*(256kb+2p+q)+k
    T2 = np.concatenate([cb, sb], axis=0)                      # [128 m, NB j]
    W = np.ascontiguousarray(T2.T)                             # [NB j, 128 m]
    W = W.reshape(KC, 128, 2, NPL, 128).transpose(1, 0, 3, 2, 4)
    W = W / np.asarray(BITVAL, np.float32)[None, None, :, None, None]
    v_host = W.astype(f8np)                                    # [128,KC,NPL,2,128]

    in_maps = []
    for c in range(NCORES):
        sl = slice(c * NI, (c + 1) * NI)
        A = m_u8[sl]                                           # [NI i, NB j]
        # pack 4 j's per byte at bits 3..6: byte[i, jb] = sum_k A[i,4jb+k]<<(3+k)
        A4 = A.reshape(NI, NJB, NPL)
        P = (
            (A4[:, :, 0] << 3) | (A4[:, :, 1] << 4)
            | (A4[:, :, 2] << 5) | (A4[:, :, 3] << 6)
        ).astype(np.uint8)                                     # [NI, NJB]
        pk_host = (
            np.ascontiguousarray(P.reshape(NI, KC, 128, 2).transpose(2, 1, 3, 0))
            .reshape(128, TK, NI)
            .view(np.uint32)
        )                                                      # [128, TK, NIW]
        in_maps.append({"pk": pk_host, "v": v_host})
    return in_maps


def combine(outs, phases_a, coupling_mask):
    pa = np.asarray(phases_a, dtype=np.float32)
    ca, sa = np.cos(pa), np.sin(pa)                            # [B, NA]
    real = np.zeros(B, np.float64)
    imag = np.zeros(B, np.float64)
    for c in range(NCORES):
        sl = slice(c * NI, (c + 1) * NI)
        y = np.asarray(outs[c]).astype(np.float32)             # [128 m, NI i]
        yt, yb_ = y[:B], y[B:]                                 # cb-part, sb-part
        cac, sac = ca[:, sl], sa[:, sl]                        # [B, NI]
        real += np.einsum('bi,bi->b', yt, cac, dtype=np.float64)
        real += np.einsum('bi,bi->b', yb_, sac, dtype=np.float64)
        imag += np.einsum('bi,bi->b', yt, sac, dtype=np.float64)
        imag -= np.einsum('bi,bi->b', yb_, cac, dtype=np.float64)
    n_pairs = max(float(np.count_nonzero(np.asarray(coupling_mask))), 1.0)
    return (np.sqrt(real * real + imag * imag) / n_pairs).astype(np.float32)


_prog_cache: list = []


def kernel(phases_a, phases_b, coupling_mask):
    in_maps = prep_inputs(phases_a, phases_b, coupling_mask)
    if not _prog_cache:
        _prog_cache.append(build_program())
    res = run_bass_kernel_spmd(_prog_cache[0], in_maps, core_ids=list(range(NCORES)))
    return combine([r["y"] for r in res.results], phases_a, coupling_mask)
